# revision 40
# baseline (speedup 1.0000x reference)
"""Bass/Trainium2 kernel for nn_BoundaryLoss (8-core data-parallel), v3.

loss = mean( w * ce ) over (B=16, H=360, W=640), where
  ce = logsumexp_c(pred) - pred[target]   (C=7)
  w  = 10 if the 17-tap ellipse window around the pixel is NOT constant
       (cv2 border semantics = replicate clamp here), else 1.

Per core: 2 images, rows in 3 groups (124/124/112); both images ride
side-by-side in the free dim so every engine op covers 2 images.

Per row-group (R rows):
  t16 [nin,2*644] fp16 target + halo rows + replicate pads   (cast DMA)
  P   [R,2*4480]  fp16 pred (one DMA per image)              (cast DMA)
  E = exp(P-1) -> fp8                                        (Act)
  boundary: ONE integer-hash 17-tap conv (random signed prime weights,
    center extra -sum(w)); window constant => hash == 0 exactly (integer
    arithmetic, exact in fp16 weights / f32 PSUM).  Non-constant windows
    hash to 0 with p ~ 1e-3-1e-4 -> ~4e-4 relative loss shift (tol 2e-2).
  m_inv = (hash == 0)  (Pool ts per image, accum -> Sum m_inv)
  S = sum_c E  (fp8 DoubleRow c-pair matmuls + single c=6)
  lse = Ln(S)  (Act)  [= logsumexp - 1]
  M: 7x tensor_scalar is_equal (DVE 4x mode); MP = M*P in place (2x tt)
  PK = sum_c MP  (PE fp16 identity matmuls)  [= pred[target]]
  d  = lse - PK  (Pool stt, accum -> Sum d)
  md = m_inv * d (DVE stt, accum -> Sum m_inv*d)
Host: loss = sum_cores[10*Sd - 9*Smd + 10*N_core - 9*Sm_inv] / N_total
      (+N folds the exp(x-1) shift: ce = d + 1.)
"""

import sys

for _p in ("/opt/trn_rl_repo",):
    if _p not in sys.path:
        sys.path.insert(0, _p)

import numpy as np
import ml_dtypes

import bass_rust
import concourse.bass as bass
import concourse.mybir as mybir
from concourse.ap import AP as APClass
from concourse.tile import TileContext
from concourse import bass_utils

F32 = mybir.dt.float32
F16 = mybir.dt.float16
F8 = mybir.dt.float8e4
I32 = mybir.dt.int32
PM = mybir.MatmulPerfMode

B_PER_CORE = 2
H, W, C = 360, 640, 7
GROUPS = [(0, 124, 0), (124, 124, 1), (248, 112, 2)]  # (row0, rows, variant)
WP = W + 4
NACC = 24

VERT = {0: [-2, -1, 0, 1, 2], -1: [-1, 0, 1], 1: [-1, 0, 1],
        -2: [-1, 0, 1], 2: [-1, 0, 1]}
TAPS = [(dy, dx) for dx, dys in VERT.items() for dy in dys]
DXS = [-2, -1, 0, 1, 2]
BANDW = 124
# cw16 layout: 15 conv bands (variant x dx) + identity
CW16_BLOCKS = len(GROUPS) * len(DXS) + 1
CW16_COLS = CW16_BLOCKS * BANDW


def _build_convw():
    """cw16 [128, CW16_COLS] fp16: hash-conv bands + identity (PK matmuls).
    cw8 [128, 2, 124] fp8: DoubleRow identity pair (S matmuls)."""
    rng = np.random.default_rng(20260808)
    primes = np.array([3, 7, 11, 19, 23, 31, 43, 53], dtype=np.int64)
    w = {t: int(primes[rng.integers(0, len(primes))]) * int(rng.choice([-1, 1]))
         for t in TAPS}
    wc = dict(w)
    wc[(0, 0)] = w[(0, 0)] - sum(w.values())

    w16 = np.zeros((128, CW16_BLOCKS, BANDW), dtype=np.float32)
    for vi, (r0, R, _v) in enumerate(GROUPS):
        in_r0 = max(r0 - 2, 0)
        for di, dx in enumerate(DXS):
            blk = vi * len(DXS) + di
            for j in range(R):
                for dy in VERT[dx]:
                    rr = min(max(r0 + j + dy, 0), H - 1)
                    w16[rr - in_r0, blk, j] += wc[(dy, dx)]
    for k in range(BANDW):
        w16[k, CW16_BLOCKS - 1, k] = 1.0
    assert np.abs(w16).max() <= 2048, "fp16 integer exactness"
    cw16 = np.ascontiguousarray(
        w16.reshape(128, CW16_COLS).astype(np.float16))

    w8 = np.zeros((128, 2, BANDW), dtype=np.float32)
    for k in range(BANDW):
        w8[k, 0, k] = 1.0
        w8[k, 1, k] = 1.0
    cw8 = np.ascontiguousarray(
        w8.reshape(128, 2 * BANDW).astype(ml_dtypes.float8_e4m3fn))
    return cw16, cw8


def split_multiwait_drains(nc, max_waits=1):
    """This walrus build rejects >1 sync-waits on CTRL-class instructions
    (the Tile end-of-kernel drain).  Split extra waits into preceding
    single-wait EventSemaphore instructions on the same engine."""
    fn = nc.m.functions[0]
    for bb in fn.blocks:
        for inst in list(bb.instructions):
            si = inst.sync_info
            if si is None or len(si.on_wait) <= max_waits:
                continue
            waits = list(si.on_wait)
            keep, extra = waits[:max_waits], waits[max_waits:]
            new_insts = []
            for k, wt in enumerate(extra):
                es = mybir.InstEventSemaphore(
                    name=f"{inst.name}-waitsplit-{k}", ins=[], outs=[])
                es.engine = inst.engine
                es.sync_info = bass_rust.SyncInfo(on_wait=[wt], on_update=[])
                nc.register_instruction(es, overwrite=True)
                new_insts.append(es)
            inst.sync_info = bass_rust.SyncInfo(
                on_wait=keep, on_update=list(si.on_update))
            pos = [i.name for i in bb.instructions].index(inst.name)
            for k, es in enumerate(new_insts):
                bb.instructions.insert(pos + k, es)


def _chunks(lo, hi, step=512):
    out = []
    while lo < hi:
        nxt = min(hi, (lo // step + 1) * step)
        out.append((lo, nxt))
        lo = nxt
    return out


def _pair_view(v2d, stride):
    """[P, cn] contiguous 2-D AP -> [P, 2, cn] with the two tiles `stride`
    elements apart (DoubleRow rhs)."""
    ap = [list(p) for p in v2d.ap]
    assert len(ap) == 2, ap
    return APClass(tensor=v2d.tensor, offset=v2d.offset,
                   ap=[ap[0], [stride, 2], ap[1]])


class _Dg:
    def __init__(self, gi):
        self.gi = gi
        self.r0, self.R, self.var = GROUPS[gi]
        self.in_r0 = max(self.r0 - 2, 0)
        self.in_r1 = min(self.r0 + self.R + 2, H)
        self.n_in = self.in_r1 - self.in_r0
        self.po = self.r0 - self.in_r0  # partition offset of center rows


def emit_loads(nc, io, sm, aps, dg):
    pred, target = aps["pred"], aps["target"]
    # P first: it gates exp, the critical chain.  dg0 loads per image so the
    # first image's exp starts early; later groups in one DMA (b and c merge:
    # b-stride == 7 * c-stride) to keep Pool descriptor work low.
    dg.P = io.tile([128, 2 * C * W], F16, tag="P")
    if dg.gi == 0 or not FLAGS["p_merge"]:
        for b in range(2):
            nc.gpsimd.dma_start(
                out=dg.P[:dg.R, b * C * W:(b + 1) * C * W].rearrange(
                    "p (c w) -> p c w", c=C),
                in_=pred[b, :, dg.r0:dg.r0 + dg.R, :].rearrange(
                    "c r w -> r c w"))
    else:
        nc.gpsimd.dma_start(
            out=dg.P[:dg.R, :].rearrange("p (bc w) -> p bc w", bc=2 * C),
            in_=pred[:, :, dg.r0:dg.r0 + dg.R, :].rearrange(
                "b c r w -> r (b c) w"))
    dg.t16 = sm.tile([128, 2 * WP], F16, tag="t16")
    nc.gpsimd.dma_start(
        out=dg.t16[:dg.n_in, :].rearrange(
            "p (b wp) -> p b wp", b=2)[:, :, 2:2 + W],
        in_=target[:, dg.in_r0:dg.in_r1, :].rearrange("b r w -> r b w"))
    # engines cannot read at a partition offset, so the mask path gets its
    # own center-rows copy of target
    dg.t16c = sm.tile([128, 2 * W], F16, tag="t16c")
    nc.gpsimd.dma_start(
        out=dg.t16c[:dg.R, :].rearrange("p (b w) -> p b w", b=2),
        in_=target[:, dg.r0:dg.r0 + dg.R, :].rearrange("b r w -> r b w"))


def emit_head(nc, pools, aps, dg):
    io, sm, cvp, spp = pools
    cw16 = aps["cw16"]
    alu = mybir.AluOpType
    AF = mybir.ActivationFunctionType
    R, n_in, gi, po = dg.R, dg.n_in, dg.gi, dg.po

    # horizontal replicate pads (DVE, tiny)
    t16v = dg.t16.rearrange("p (b wp) -> p b wp", b=2)
    for b in range(2):
        nc.vector.tensor_copy(t16v[:n_in, b, 0:2],
                              t16v[:n_in, b, 2:3].broadcast_to([n_in, 2]))
        nc.vector.tensor_copy(t16v[:n_in, b, W + 2:W + 4],
                              t16v[:n_in, b, W + 1:W + 2].broadcast_to([n_in, 2]))

    # E = exp(P - 1) -> fp8 (dg0: per-image ops to chase the P DMA halves)
    dg.E = io.tile([128, 2 * C * W], F16, tag="E")
    splits = ((0, C * W), (C * W, 2 * C * W)) \
        if (gi == 0 and FLAGS["exp0_split"]) else ((0, 2 * C * W),)
    for (s0, s1) in splits:
        nc.scalar.activation(dg.E[:R, s0:s1], dg.P[:R, s0:s1], AF.Exp,
                             bias=aps["neg1"][:R, 0:1], scale=1.0)

    # hash conv (PE fp16) + m_inv, per image
    cw16v = cw16.rearrange("p (blk j) -> p blk j", blk=CW16_BLOCKS)
    dg.m_inv = sm.tile([128, 2 * W], F16, tag="m_inv")
    for b in range(2):
        rr = cvp.tile([128, W], F32, tag="rr")
        for (o0, o1) in _chunks(0, W):
            cn = o1 - o0
            for di, dx in enumerate(DXS):
                blk = dg.var * len(DXS) + di
                col = b * WP + 2 + dx + o0
                nc.tensor.matmul(rr[:R, o0:o1], cw16v[:n_in, blk, :R],
                                 dg.t16[:n_in, col:col + cn],
                                 start=(di == 0), stop=(di == len(DXS) - 1))
        nc.vector.tensor_scalar(
            out=dg.m_inv[:R, b * W:(b + 1) * W], in0=rr[:R, :],
            scalar1=0.0, scalar2=None, op0=alu.is_equal)

    # masks (7x ts is_equal, 4x mode), then MP = M*P in place
    dg.M = io.tile([128, 2 * C * W], F16, tag="M")
    Mv = dg.M.rearrange("p (b c w) -> p b c w", b=2, c=C)
    tc16 = dg.t16c[:R, :].rearrange("p (b w) -> p b w", b=2)
    for c in range(C):
        eng = nc.gpsimd if c < FLAGS["mask_pool_n"] else nc.vector
        eng.tensor_scalar(out=Mv[:R, :, c, :], in0=tc16,
                          scalar1=float(c), scalar2=None, op0=alu.is_equal)
    # MP = M*P in place; split per image on the last group so PK can start
    # on image 0 while image 1's product still runs
    pc = FLAGS["mp_pool_cols"]
    if pc:
        nc.gpsimd.tensor_tensor(out=dg.M[:R, 2 * C * W - pc:],
                                in0=dg.M[:R, 2 * C * W - pc:],
                                in1=dg.P[:R, 2 * C * W - pc:], op=alu.mult)
    if gi == len(GROUPS) - 1 and FLAGS["mp_split_last"]:
        for b in range(2):
            s0, s1 = b * C * W, (b + 1) * C * W
            s1 = min(s1, 2 * C * W - pc)
            if s0 < s1:
                nc.vector.tensor_tensor(out=dg.M[:R, s0:s1],
                                        in0=dg.M[:R, s0:s1],
                                        in1=dg.P[:R, s0:s1], op=alu.mult)
    else:
        nc.vector.tensor_tensor(out=dg.M[:R, :2 * C * W - pc],
                                in0=dg.M[:R, :2 * C * W - pc],
                                in1=dg.P[:R, :2 * C * W - pc], op=alu.mult)

    # S = sum_c E (fp8 DR pairs + single c=6); PK = sum_c MP (fp16)
    dg.S = spp.tile([128, 2 * W], F32, tag="S")
    dg.PK = spp.tile([128, 2 * W], F32, tag="PK")
    id16 = cw16v[:R, CW16_BLOCKS - 1, :R]
    def _emit_pk():
        for b in range(2):
            for (o0, o1) in _chunks(b * W, (b + 1) * W):
                rel0 = o0 - b * W
                cn = o1 - o0
                for c in range(C):
                    col = b * C * W + c * W + rel0
                    nc.tensor.matmul(dg.PK[:R, o0:o1], id16,
                                     dg.M[:R, col:col + cn],
                                     start=(c == 0), stop=(c == C - 1))

    def _emit_s():
        for b in range(2):
            for (o0, o1) in _chunks(b * W, (b + 1) * W):
                rel0 = o0 - b * W
                cn = o1 - o0
                for c in range(C):
                    col = b * C * W + c * W + rel0
                    nc.tensor.matmul(dg.S[:R, o0:o1], id16,
                                     dg.E[:R, col:col + cn],
                                     start=(c == 0), stop=(c == C - 1))

    # On the last group close the d->md tail as early as possible: MP is
    # ready before E there, so PK goes first.  Earlier groups: S first so
    # ln/exp pipelining on Act is not blocked behind MP.
    if gi == len(GROUPS) - 1 and FLAGS["pk_last_first"]:
        _emit_pk()
        _emit_s()
    else:
        _emit_s()
        _emit_pk()


def emit_tail(nc, pools, aps, dg):
    io, sm, cvp, spp = pools
    alu = mybir.AluOpType
    AF = mybir.ActivationFunctionType
    R, gi = dg.R, dg.gi

    last = gi == len(GROUPS) - 1
    import math
    lse = sm.tile([128, 2 * W], F16, tag="lse")
    # ln(e*S) = logsumexp: folds the exp(x-1) shift back in
    nc.scalar.activation(lse[:R, :], dg.S[:R, :], AF.Ln, scale=math.e)
    d = sm.tile([128, 2 * W], F16, tag="d")
    mdj = sm.tile([128, 2 * W], F16, tag="mdj")
    # d (reads PSUM) on DVE; md (all-SBUF) on Pool.  Per-image split on the
    # last group, where the d->md chain closes the kernel.
    bsplits = ((0, W), (W, 2 * W)) if (last and FLAGS["tail_split"]) \
        else ((0, 2 * W),)
    for bi, (s0, s1) in enumerate(bsplits):
        s = slice(s0, s1)
        nc.vector.scalar_tensor_tensor(
            out=d[:R, s], in0=dg.PK[:R, s], scalar=-1.0, in1=lse[:R, s],
            op0=alu.mult, op1=alu.add,
            accum_out=aps["a_d"][:R, 2 * gi + bi:2 * gi + bi + 1])
        nc.vector.scalar_tensor_tensor(
            out=mdj[:R, s], in0=dg.m_inv[:R, s], scalar=0.0, in1=d[:R, s],
            op0=alu.bypass, op1=alu.mult,
            accum_out=aps["a_md"][:R, 2 * gi + bi:2 * gi + bi + 1])


FLAGS = {"cw_split": False, "pk_last_first": False, "tail_split": True,
         "exp0_split": True, "p_merge": False, "mp_split_last": False,
         "memset_pool": False, "mask_pool_n": 0, "mp_pool_cols": 0,
         "md_last_dve": False}


def build_nc(io_bufs=2, sm_bufs=2):
    nc = bass.Bass()
    pred = nc.dram_tensor("pred", [B_PER_CORE, C, H, W], F32,
                          kind="ExternalInput")
    target = nc.dram_tensor("target", [B_PER_CORE, H, W], I32,
                            kind="ExternalInput")
    convw16 = nc.dram_tensor("convw16", [128, CW16_COLS], F16,
                             kind="ExternalInput")
    acc_out = nc.dram_tensor("acc", [128, NACC], F32, kind="ExternalOutput")

    with TileContext(nc, pool_alloc_mode="stack") as tc:
        with (
            tc.tile_pool(name="io", bufs=io_bufs) as io,
            tc.tile_pool(name="sm", bufs=sm_bufs) as sm,
            tc.tile_pool(name="cv", bufs=1, space="PSUM") as cvp,
            tc.tile_pool(name="sp", bufs=1, space="PSUM") as spp,
            tc.tile_pool(name="const", bufs=1) as cpool,
        ):
            a_d = cpool.tile([128, 8], F32)
            nc.vector.memset(a_d[:, :], 0.0)
            a_md = cpool.tile([128, 8], F32)
            nc.vector.memset(a_md[:, :], 0.0)
            neg1 = cpool.tile([128, 1], F32)
            nc.vector.memset(neg1[:, :], -1.0)
            cw16_sb = cpool.tile([128, CW16_COLS], F16)
            aps = {"pred": pred.ap(), "target": target.ap(),
                   "cw16": cw16_sb, "neg1": neg1,
                   "a_d": a_d, "a_md": a_md}
            pools = (io, sm, cvp, spp)

            dgs = [_Dg(i) for i in range(len(GROUPS))]
            emit_loads(nc, io, sm, aps, dgs[0])
            # weights after the first P/t16 loads, split per variant block so
            # no single transfer delays P on the shared DMA engines
            nbv = len(DXS) * BANDW
            if FLAGS["cw_split"]:
                nc.sync.dma_start(out=cw16_sb[:, 15 * BANDW:],
                                  in_=convw16.ap()[:, 15 * BANDW:])
                for vi in range(len(GROUPS)):
                    nc.sync.dma_start(
                        out=cw16_sb[:, vi * nbv:(vi + 1) * nbv],
                        in_=convw16.ap()[:, vi * nbv:(vi + 1) * nbv])
            else:
                nc.sync.dma_start(out=cw16_sb[:, :], in_=convw16.ap())
            emit_loads(nc, io, sm, aps, dgs[1])
            for g in range(len(dgs)):
                emit_head(nc, pools, aps, dgs[g])
                if g + 2 < len(dgs):
                    emit_loads(nc, io, sm, aps, dgs[g + 2])
                if g > 0:
                    emit_tail(nc, pools, aps, dgs[g - 1])
            emit_tail(nc, pools, aps, dgs[-1])

            nc.sync.dma_start(out=acc_out.ap()[:, 8:16], in_=a_d[:, :])
            nc.sync.dma_start(out=acc_out.ap()[:, 16:24], in_=a_md[:, :])

    split_multiwait_drains(nc)
    return nc


_CACHED = {}


def _get_nc():
    if "nc" not in _CACHED:
        _CACHED["nc"] = build_nc()
        _CACHED["convw16"], _CACHED["convw8"] = _build_convw()
    return _CACHED["nc"], _CACHED["convw16"]


def combine_acc(acc_tiles):
    n_core = B_PER_CORE * H * W
    total = 0.0
    for a in acc_tiles:
        a = a.astype(np.float64)
        sd = a[:, 8:16].sum()
        smd = a[:, 16:24].sum()
        total += 10.0 * sd - 9.0 * smd
    return np.float32(total / (8 * n_core))


def kernel(pred, target):
    nc, convw16 = _get_nc()
    n_cores = 8
    in_maps = []
    for i in range(n_cores):
        in_maps.append({
            "pred": np.ascontiguousarray(pred[2 * i:2 * i + 2]),
            "target": np.ascontiguousarray(target[2 * i:2 * i + 2]),
            "convw16": convw16,
        })
    res = bass_utils.run_bass_kernel_spmd(nc, in_maps,
                                          core_ids=list(range(n_cores)))
    return combine_acc([r["acc"] for r in res.results])


# revision 52
# speedup vs baseline: 1.0881x; 1.0881x over previous
"""Bass/Trainium2 kernel for nn_BoundaryLoss (8-core data-parallel).

loss = mean( ce * weight ) over (B=16, H=360, W=640) pixels, where
  ce     = logsumexp_c(pred) - pred[target]          (C=7)
  weight = 10 if 5x5-ellipse window around the pixel is NOT constant else 1
           (morphological gradient > 0, cv2 border-ignoring semantics)

Sharding: pure data parallel, 2 images per NeuronCore.  Each core emits a
[128, 64] f32 accumulator tile holding per-partition partial sums
(w = 1 + 9*boundary):
  cols  0..11 : sum(w * lse)     per (group, half)
  cols 32..43 : sum(w * picked)  per (group, half)
Host: loss = ( S_wlse - S_wpk ) / (B*H*W)  -- the tiny 8-way combine is the
all-reduce from the sharding hint, done on host since kernel() returns the
full output anyway.

Morphology is computed exactly with the variance trick: the window is
constant  <=>  17*S2 == S1^2  where S1 = sum(t), S2 = sum(t^2) over the
17-tap ellipse with replicate clamping at borders (replicate-clamped taps
always fall inside the in-image window, so this matches cv2's
border-ignoring max/min).  All quantities are small integers -> exact in
bf16 matmuls + fp32 PSUM.
"""

import sys

for _p in ("/opt/trn_rl_repo",):
    if _p not in sys.path:
        sys.path.insert(0, _p)

import numpy as np
import ml_dtypes

import bass_rust
import concourse.bass as bass
import concourse.mybir as mybir
from concourse.tile import TileContext
from concourse import bass_utils

F32 = mybir.dt.float32
BF16 = mybir.dt.float16  # fp16: 10-bit mantissa, exact ints 0..2048, exp(P)<=~200 safe
I32 = mybir.dt.int32

B_PER_CORE = 2
H, W, C = 360, 640, 7
# (row0, rows, variant): variant 0=top-clamped, 1=interior, 2=bottom-clamped
GROUPS = [(0, 124, 0), (124, 124, 1), (248, 112, 2)]
NV = 3  # conv variants stored in convw
WPAD = W + 4
NCOL = 64  # acc tile columns

# ellipse 5x5 taps grouped by dx -> vertical dy list
VERT = {0: [-2, -1, 0, 1, 2], -1: [-1, 0, 1], 1: [-1, 0, 1],
        -2: [-1, 0, 1], 2: [-1, 0, 1]}
DXS = [-2, -1, 0, 1, 2]


def _build_convw():
    """[16, 128, 124] bf16: per group-position (3) x dx (5) banded vertical
    conv lhsT with border clamping baked in; slot 15 = identity."""
    w = np.zeros((16, 128, 124), dtype=np.float32)
    seen = {}
    for (r0, R, v) in GROUPS:
        if v in seen:
            continue
        seen[v] = True
        in_r0 = max(r0 - 2, 0)
        in_r1 = min(r0 + R + 2, H)
        for dxi, dx in enumerate(DXS):
            for j in range(R):
                for dy in VERT[dx]:
                    rr = min(max(r0 + j + dy, 0), H - 1)
                    k = rr - in_r0
                    assert 0 <= k < in_r1 - in_r0 <= 128
                    w[v * 5 + dxi, k, j] += 1.0
    for k in range(124):
        w[15, k, k] = 1.0
    return np.ascontiguousarray(
        w.transpose(1, 0, 2).reshape(128, 16 * 124)).astype(np.float16)


def split_multiwait_drains(nc, max_waits=1):
    """This walrus build rejects >1 sync-waits on CTRL-class instructions
    (the Tile end-of-kernel drain).  Split extra waits into preceding
    single-wait EventSemaphore instructions on the same engine."""
    fn = nc.m.functions[0]
    for bb in fn.blocks:
        for inst in list(bb.instructions):
            si = inst.sync_info
            if si is None or len(si.on_wait) <= max_waits:
                continue
            waits = list(si.on_wait)
            keep, extra = waits[:max_waits], waits[max_waits:]
            new_insts = []
            for k, wt in enumerate(extra):
                es = mybir.InstEventSemaphore(
                    name=f"{inst.name}-waitsplit-{k}", ins=[], outs=[])
                es.engine = inst.engine
                es.sync_info = bass_rust.SyncInfo(on_wait=[wt], on_update=[])
                nc.register_instruction(es, overwrite=True)
                new_insts.append(es)
            inst.sync_info = bass_rust.SyncInfo(
                on_wait=keep, on_update=list(si.on_update))
            pos = [i.name for i in bb.instructions].index(inst.name)
            for k, es in enumerate(new_insts):
                bb.instructions.insert(pos + k, es)


def _emit_group(nc, tc, pools, aps, b, gi):
    """Emit all work for (image b, row-group gi)."""
    r0, R, var = GROUPS[gi]
    in_r0 = max(r0 - 2, 0)
    in_r1 = min(r0 + R + 2, H)
    n_in = in_r1 - in_r0
    g = b * len(GROUPS) + gi  # global group index

    pred, target, convw_sb, acc = aps[:4]
    io, sm, ps, psm = pools

    alu = mybir.AluOpType
    AF = mybir.ActivationFunctionType

    # ---- loads (t first: small DMAs unblock DVE/PE while P streams) ----
    t_pad = sm.tile([128, WPAD], BF16, tag="t_pad")
    nc.gpsimd.dma_start(out=t_pad[:n_in, 2:2 + W],
                        in_=target[b, in_r0:in_r1, :])
    if var == 0:
        # top group: rows start at partition 0 of t_pad, reuse it directly
        t_ctr = t_pad[:, 2:2 + W]
    else:
        t_ctr = sm.tile([128, W], BF16, tag="t_ctr")
        nc.gpsimd.dma_start(out=t_ctr[:R, :], in_=target[b, r0:r0 + R, :])

    P = io.tile([128, C * W], BF16, tag="P")
    nc.gpsimd.dma_start(
        out=P[:R, :],
        in_=pred[b, :, r0:r0 + R, :].rearrange("c r w -> r c w"))
    # horizontal replicate pad (2 cols each side)
    nc.vector.tensor_copy(t_pad[:n_in, 0:2],
                          t_pad[:n_in, 2:3].broadcast_to([n_in, 2]))
    nc.vector.tensor_copy(t_pad[:n_in, W + 2:W + 4],
                          t_pad[:n_in, W + 1:W + 2].broadcast_to([n_in, 2]))

    t2_pad = sm.tile([128, WPAD], BF16, tag="t2_pad")
    nc.scalar.square(t2_pad[:n_in, :], t_pad[:n_in, :])

    # ---- CE: mask / exp / reduce ----------------------------------------
    MG = io.tile([128, C * W], BF16, tag="MG")
    for c in range(C):
        sl = slice(c * W, (c + 1) * W)
        nc.vector.tensor_scalar(out=MG[:R, sl], in0=t_ctr[:R, :],
                                scalar1=float(c), scalar2=None,
                                op0=alu.is_equal)
    E = io.tile([128, C * W], BF16, tag="E")
    nc.scalar.activation(E[:R, :C * W // 2], P[:R, :C * W // 2], AF.Exp)
    nc.scalar.activation(E[:R, C * W // 2:], P[:R, C * W // 2:], AF.Exp)

    MP = io.tile([128, C * W], BF16, tag="MP")
    nc.vector.tensor_mul(MP[:R, :], MG[:R, :], P[:R, :])

    idw = convw_sb[:R, 15 * 124:15 * 124 + R]
    HW_ = W // 2  # 320-col halves: S/PK PSUM tiles are one bank each

    # morphology: S1/S2 ellipse conv on PE, full-width 2-bank PSUM tiles
    # (bufs=1 pool: their consumers below are fast, so serialization is
    # cheap, and full-width halves the fixed cost of square/cmp/W ops)
    S2_ps = psm.tile([128, W], F32, tag="S2")
    S1_ps = psm.tile([128, W], F32, tag="S1")
    for dxi, dx in enumerate(DXS):
        co = (var * 5 + dxi) * 124
        lhsT = convw_sb[:n_in, co:co + R]
        st, sp = (dxi == 0), (dxi == 4)
        for (c0, c1) in ((0, 512), (512, W)):
            nc.tensor.matmul(S2_ps[:R, c0:c1], lhsT,
                             t2_pad[:n_in, 2 + dx + c0:2 + dx + c1],
                             start=st, stop=sp)
            nc.tensor.matmul(S1_ps[:R, c0:c1], lhsT,
                             t_pad[:n_in, 2 + dx + c0:2 + dx + c1],
                             start=st, stop=sp)

    S1sq = sm.tile([128, W], F32, tag="S1sq")
    nc.scalar.square(S1sq[:R, :], S1_ps[:R, :])
    m = sm.tile([128, W], BF16, tag="m")
    nc.vector.scalar_tensor_tensor(
        out=m[:R, :], in0=S2_ps[:R, :], scalar=17.0, in1=S1sq[:R, :],
        op0=alu.mult, op1=alu.is_gt)
    Wt = sm.tile([128, W], BF16, tag="Wt")
    nc.vector.tensor_scalar(out=Wt[:R, :], in0=m[:R, :], scalar1=9.0,
                            scalar2=1.0, op0=alu.mult, op1=alu.add)

    for h in range(2):
        hs = slice(h * HW_, (h + 1) * HW_)
        S_ps = ps.tile([128, HW_], F32, tag="S")
        PK_ps = ps.tile([128, HW_], F32, tag="PK")
        for c in range(C):
            sl = slice(c * W + h * HW_, c * W + (h + 1) * HW_)
            st, sp = (c == 0), (c == C - 1)
            nc.tensor.matmul(S_ps[:R, :], idw, E[:R, sl], start=st, stop=sp)
            nc.tensor.matmul(PK_ps[:R, :], idw, MP[:R, sl], start=st, stop=sp)

        lse = sm.tile([128, HW_], BF16, tag="lse")
        nc.scalar.activation(lse[:R, :], S_ps[:R, :], AF.Ln)

        junk1 = sm.tile([128, HW_], BF16, tag="junk1")
        nc.vector.scalar_tensor_tensor(
            out=junk1[:R, :], in0=Wt[:R, hs], scalar=0.0, in1=lse[:R, :],
            op0=alu.bypass, op1=alu.mult,
            accum_out=acc[:R, 2 * g + h:2 * g + h + 1])
        junk2 = sm.tile([128, HW_], BF16, tag="junk2")
        nc.vector.scalar_tensor_tensor(
            out=junk2[:R, :], in0=Wt[:R, hs], scalar=0.0, in1=PK_ps[:R, :],
            op0=alu.bypass, op1=alu.mult,
            accum_out=acc[:R, 32 + 2 * g + h:32 + 2 * g + h + 1])


def build_nc(io_bufs=3, sm_bufs=7, ps_bufs=2, pool_mode="stack"):
    nc = bass.Bass()
    pred = nc.dram_tensor("pred", [B_PER_CORE, C, H, W], F32,
                          kind="ExternalInput")
    target = nc.dram_tensor("target", [B_PER_CORE, H, W], I32,
                            kind="ExternalInput")
    convw = nc.dram_tensor("convw", [128, 16 * 124], BF16,
                           kind="ExternalInput")
    acc_out = nc.dram_tensor("acc", [128, NCOL], F32, kind="ExternalOutput")

    with TileContext(nc, pool_alloc_mode=pool_mode) as tc:
        with (
            tc.tile_pool(name="io", bufs=io_bufs) as io,
            tc.tile_pool(name="sm", bufs=sm_bufs) as sm,
            tc.tile_pool(name="ps", bufs=ps_bufs, space="PSUM") as ps,
            tc.tile_pool(name="psm", bufs=1, space="PSUM") as psm,
            tc.tile_pool(name="const", bufs=1) as cpool,
        ):
            convw_sb = cpool.tile([128, 16 * 124], BF16)
            nc.sync.dma_start(out=convw_sb[:, :], in_=convw.ap())
            acc = cpool.tile([128, NCOL], F32)
            nc.vector.memset(acc[:, :], 0.0)
            aps = (pred.ap(), target.ap(), convw_sb, acc)
            for b in range(B_PER_CORE):
                for gi in range(len(GROUPS)):
                    _emit_group(nc, tc, (io, sm, ps, psm), aps, b, gi)

            nc.sync.dma_start(out=acc_out.ap(), in_=acc[:, :])

    split_multiwait_drains(nc)
    return nc


_CACHED = {}


def _get_nc():
    if "nc" not in _CACHED:
        _CACHED["nc"] = build_nc()
        _CACHED["convw"] = _build_convw()
    return _CACHED["nc"], _CACHED["convw"]


def combine_acc(acc_tiles):
    """acc_tiles: list of [128, 64] f32 -> scalar loss (f32)."""
    ng = 2 * B_PER_CORE * len(GROUPS)
    s_wlse = s_wpk = 0.0
    for a in acc_tiles:
        a = a.astype(np.float64)
        s_wlse += a[:, 0:ng].sum()
        s_wpk += a[:, 32:32 + ng].sum()
    n = 16 * H * W
    loss = (s_wlse - s_wpk) / n
    return np.float32(loss)


def kernel(pred, target):
    nc, convw = _get_nc()
    n_cores = 8
    in_maps = []
    for i in range(n_cores):
        in_maps.append({
            "pred": np.ascontiguousarray(pred[2 * i:2 * i + 2]),
            "target": np.ascontiguousarray(target[2 * i:2 * i + 2]),
            "convw": convw,
        })
    res = bass_utils.run_bass_kernel_spmd(nc, in_maps,
                                          core_ids=list(range(n_cores)))
    return combine_acc([r["acc"] for r in res.results])

